# revision 8
# baseline (speedup 1.0000x reference)
"""Trainium2 Bass kernel for nn_DebugBertLayer_87093346828840.

Key observation: the reference overwrites q/k/v with the constant 0.01, so
softmax(scores) is uniform and ctx == 0.01 everywhere.  Hence
    attn_out = LN1(hidden + cvec),   cvec = 0.01 * Wo.sum(axis=1) + bo
and the only real device work is the FFN:
    out = LN2( gelu(attn_out @ Wi.T + bi) @ Wf.T + bf + attn_out )

Sharding: pure data-parallel over the 8192 tokens -> 1024 tokens/core on 8
NeuronCores, no collectives.

Matmuls run in fp8e4 (e4m3) with perf_mode=DoubleRow: 2 fp8 weights/PE cell,
2 MACs/cycle, contraction 256/instruction -> ~1.8x the bf16 PE throughput.
Scale plumbing (all folded into existing ops, zero extra passes):
  - weights quantized at x256 host-side (keeps |w|~0.02 out of fp8 subnormals)
  - LN1's rsqrt is scaled by 256, so a_tiles hold 256*attn_out (fp32)
  - the fp8 cast of a_tiles uses scale 1/8 -> activations at x32
  - mm1 psum = 8192*h_pre; ACT Gelu drains it with scale=1/8192 -> h exact,
    written straight to fp8 (scale 1)
  - mm2 psum = 256*(h@Wf.T); the residual add uses the pre-scaled a_tiles and
    LayerNorm is scale-invariant, so LN2 absorbs the x256 for free.
DoubleRow pairing: contraction index (p, i) -> feature 2p+i (adjacent pairs),
so a [128,128]-uint16 DMA-xbar transpose of packed fp8 pairs produces exactly
the [p][i=2][tok] moving-operand layout mm1 needs.

Cross-rep software pipelining (the timing loop runs 2 reps per For_i
iteration, ping-ponging A/B buffer generations):
  body = mm1(A) ; mm2(A) with prep(B) interleaved tile-by-tile ;
         mm1(B) ; mm2(B) with prep(A') interleaved
prep(gen, t) = x DMA + cvec add + LN1 + fp8 cast (DVE) + 3 uint16 DMA-xbar
transposes (on the ACT HWDGE ring, so they never queue behind x/y traffic on
the SP ring).  The DVE ops of prep(next) are emitted interleaved with mm2's
drain+LN2 per token tile, so the Vector engine's FIFO alternates between
draining this rep and preparing the next one; the PE goes straight from
mm2(A) into mm1(B) with zero boundary bubble.
"""

import os
import sys

for _p in ("/opt/trn_rl_repo", "/root/.axon_site/_ro/trn_rl_repo"):
    if os.path.isdir(_p) and _p not in sys.path:
        sys.path.insert(0, _p)

import numpy as np
import ml_dtypes

import concourse.bass as bass
import concourse.bacc as bacc
import concourse.tile as tile
from concourse import mybir
from concourse.bass_utils import run_bass_kernel_spmd

F32 = mybir.dt.float32
BF16 = mybir.dt.bfloat16
F8 = mybir.dt.float8e4
U16 = mybir.dt.uint16
AF = mybir.ActivationFunctionType
ALU = mybir.AluOpType
DR = mybir.MatmulPerfMode.DoubleRow
F8NP = mybir.dt.np(mybir.dt.float8e4)   # ml_dtypes.float8_e4m3

D = 768           # d_model
FF = 3072         # d_ff
NCORE = 8
TOK = 8192        # total tokens (4 x 2048)
TPC = TOK // NCORE  # 1024 tokens per core
KK = D // 256     # 3 DoubleRow k-tiles over d_model
MF = FF // 128    # 24 tiles over d_ff
JJ = FF // 256    # 12 DoubleRow ff-tiles
NT = TPC // 128   # 8 token tiles per core
HALF = TPC // 2   # 512
LN_EPS = 1e-12

S_W = 256.0       # weight quant scale
S_A = 32.0        # activation quant scale (cast scale = S_A / S_W = 1/8)

_NC_CACHE = {}
LAST_RESULTS = None
RUN_KWARGS = {}


def _ln_tile(nc, pstat, s_t, gb, apply_gb, post_scale=1.0):
    """In-place LayerNorm over the free dim (768) of s_t [128, 768] f32.

    rstd is computed on the Vector engine (bit-trick seed + 2 Newton steps)
    instead of ScalarE Sqrt: the ACT engine then only ever runs Gelu, which
    avoids ~1.3-7.6us activation-table reloads on every Sqrt<->Gelu switch.
    post_scale is folded into rstd, so the output is post_scale*LN(x).
    var+eps == var in fp32 here (var ~1 >> 1e-12), so eps is skipped.
    """
    g_b, b_b = gb
    sr = s_t.rearrange("p (n s) -> p n s", s=384)
    stats = pstat.tile([128, 2, 6], F32, tag="stats")
    for i in range(2):
        nc.vector.bn_stats(out=stats[:, i, :], in_=sr[:, i, :])
    mv = pstat.tile([128, 2], F32, tag="mv")
    nc.vector.bn_aggr(out=mv[:], in_=stats[:])
    v = mv[:, 1:2]
    rst = pstat.tile([128, 1], F32, tag="rst")
    nrt = pstat.tile([128, 1], F32, tag="nrt")
    # y0 = bitcast(0x5f3759df - (bits(v) >> 1)): ~3.4% rsqrt seed
    nc.vector.tensor_scalar(out=rst.bitcast(mybir.dt.int32)[:],
                            in0=v.bitcast(mybir.dt.int32),
                            scalar1=1, scalar2=None,
                            op0=ALU.logical_shift_right)
    nc.vector.tensor_scalar(out=rst.bitcast(mybir.dt.int32)[:],
                            in0=rst.bitcast(mybir.dt.int32)[:],
                            scalar1=-1, scalar2=0x5F3759DF,
                            op0=ALU.mult, op1=ALU.add)
    for it in range(2):  # y <- y*(1.5 - 0.5*v*y^2); 2 steps -> ~1e-6 rel
        nc.vector.tensor_mul(out=nrt[:], in0=rst[:], in1=rst[:])
        nc.vector.tensor_mul(out=nrt[:], in0=nrt[:], in1=v)
        nc.vector.tensor_scalar(out=nrt[:], in0=nrt[:], scalar1=-0.5,
                                scalar2=1.5, op0=ALU.mult, op1=ALU.add)
        if it == 1 and post_scale != 1.0:
            # fold post_scale into the last Newton step's nrt factor
            nc.vector.tensor_scalar(out=nrt[:], in0=nrt[:],
                                    scalar1=post_scale, scalar2=None,
                                    op0=ALU.mult)
        nc.vector.tensor_mul(out=rst[:], in0=rst[:], in1=nrt[:])
    nc.vector.tensor_scalar(out=s_t[:], in0=s_t[:], scalar1=mv[:, 0:1],
                            scalar2=rst[:], op0=ALU.subtract, op1=ALU.mult)
    if apply_gb:
        nc.vector.tensor_mul(out=s_t[:], in0=s_t[:], in1=g_b[:])
        nc.vector.tensor_add(out=s_t[:], in0=s_t[:], in1=b_b[:])


class _Gen:
    """One generation of prepared activations (a_tiles + transposed fp8)."""

    def __init__(self, name):
        self.name = name
        self.a_tiles = [None] * NT
        self.aT8_u16 = None
        self.aT_r = None
        self.hT = None


class _Emitter:
    def __init__(self, nc, pools, tensors, flags):
        self.nc = nc
        (self.pw, self.px, self.pbig, self.pabf, self.pstat, self.pout,
         self.ps1, self.psm) = pools
        (self.x, self.y, self.wi8_sb, self.wf8_tiles, self.cvec_b,
         self.g1_b, self.b1_b, self.g2_b, self.b2_b, self.bfv_b,
         self.bi_sb) = tensors
        self.flags = flags
        self.wi8_r = self.wi8_sb.rearrange("p (c i f) -> p c i f", c=KK, i=2)
        self.wf_r = [w.rearrange("p (i d) -> p i d", i=2)
                     for w in self.wf8_tiles]

    def gen_alloc(self, gen):
        # fp8 [p][c][tok][i]: u16 view column c*TPC+t packs the
        # adjacent-feature pair (2p, 2p+1) of double-k-tile c at token t
        aT8 = self.pbig.tile([128, KK * TPC * 2], F8, tag=f"aT{gen.name}")
        gen.aT8_u16 = aT8.bitcast(U16)   # [128, KK*TPC]
        gen.aT_r = aT8.rearrange("p (c t i) -> p c i t", c=KK, i=2)
        gen.hT = self.pbig.tile([128, MF * TPC], F8, tag=f"hT{gen.name}")

    def prep_tile(self, gen, t, x_t=None):
        """x DMA + cvec + LN1(*S_W) + fp8 cast + 3 u16 DMA transposes."""
        nc = self.nc
        if x_t is None:
            x_t = self.px.tile([128, D], F32, tag="xa")
            nc.sync.dma_start(out=x_t[:],
                              in_=self.x[t * 128:(t + 1) * 128, :])
        nc.vector.tensor_add(out=x_t[:], in0=x_t[:], in1=self.cvec_b[:])
        _ln_tile(nc, self.pstat, x_t, (self.g1_b, self.b1_b),
                 self.flags["g1b1"], post_scale=S_W)
        gen.a_tiles[t] = x_t
        a8 = self.pabf.tile([128, D], F8, tag="a8")
        nc.vector.tensor_scalar(out=a8[:], in0=x_t[:],
                                scalar1=S_A / S_W, scalar2=None,
                                op0=ALU.mult)
        a8u = a8.bitcast(U16)  # [128, 384] feature pairs
        for c in range(KK):
            dst = gen.aT8_u16[:, c * TPC + t * 128: c * TPC + (t + 1) * 128]
            # ACT's HWDGE ring: keeps transposes off the x/y SP ring
            nc.scalar.dma_start(out=dst, in_=a8u[:, c * 128:(c + 1) * 128],
                                transpose=True)

    def mm1(self, gen, hT):
        nc = self.nc
        for ph in range(2):
            off = ph * HALF
            for m in range(MF):
                ps_a = self.ps1.tile([128, HALF], F32, tag="hps")
                for c in range(KK):
                    lhsT = self.wi8_r[:, c, :, m * 128:(m + 1) * 128]
                    nc.tensor.matmul(ps_a[:], lhsT,
                                     gen.aT_r[:, c, :, off:off + HALF],
                                     start=(c == 0), stop=(c == KK - 1),
                                     perf_mode=DR)
                dst = hT[:, m * TPC + off: m * TPC + off + HALF]
                if self.flags["bi"]:
                    nc.scalar.activation(out=dst, in_=ps_a[:], func=AF.Gelu,
                                         bias=self.bi_sb[:, m:m + 1],
                                         scale=1.0 / (S_A * S_W))
                else:
                    nc.scalar.activation(out=dst, in_=ps_a[:], func=AF.Gelu,
                                         scale=1.0 / (S_A * S_W))

    def mm2(self, gen, hT, prep_cb=None):
        nc = self.nc
        # stationary hT slice [p][i=2][tok]: ff = (2*jj + i)*128 + p
        hT_r = hT.rearrange("p (j i t) -> p j i t", j=JJ, i=2)
        for t in range(NT):
            ps2 = self.psm.tile([128, D], F32, tag="psm")
            for jj in range(JJ):
                lhsT = hT_r[:, jj, :, t * 128:(t + 1) * 128]
                nc.tensor.matmul(ps2[:, 0:512], lhsT, self.wf_r[jj][:, :, 0:512],
                                 start=(jj == 0), stop=(jj == JJ - 1),
                                 perf_mode=DR)
                nc.tensor.matmul(ps2[:, 512:768], lhsT,
                                 self.wf_r[jj][:, :, 512:768],
                                 start=(jj == 0), stop=(jj == JJ - 1),
                                 perf_mode=DR)
            s_t = self.pout.tile([128, D], F32, tag="s")
            nc.vector.tensor_add(out=s_t[:], in0=ps2[:],
                                 in1=gen.a_tiles[t][:])
            if self.flags["bfv"]:
                nc.vector.tensor_add(out=s_t[:], in0=s_t[:], in1=self.bfv_b[:])
            _ln_tile(nc, self.pstat, s_t, (self.g2_b, self.b2_b),
                     self.flags["g2b2"])
            nc.sync.dma_start(out=self.y[t * 128:(t + 1) * 128, :],
                              in_=s_t[:])
            if prep_cb is not None:
                prep_cb(t)

    def rep(self, gen, nxt):
        """mm1+mm2 on `gen`; prep of `nxt` interleaved into mm2 (or None)."""
        self.mm1(gen, gen.hT)
        if nxt is not None:
            self.mm2(gen, gen.hT, prep_cb=lambda t: self.prep_tile(nxt, t))
        else:
            self.mm2(gen, gen.hT)


def _bcast_ap(handle, n):
    """AP that broadcasts a [n]-vector across 128 partitions for DMA."""
    return bass.AP(tensor=handle, offset=0, ap=[[0, 128], [1, n]])


def _build(n_reps=1, flag_key=(True, True, True, True)):
    cache_key = (n_reps, flag_key)
    if cache_key in _NC_CACHE:
        return _NC_CACHE[cache_key]
    flags = dict(zip(("g1b1", "g2b2", "bi", "bfv"), flag_key))
    nc = bacc.Bacc("TRN2", target_bir_lowering=False, debug=False,
                   num_devices=NCORE)
    x = nc.dram_tensor("x", [TPC, D], F32, kind="ExternalInput")
    wi = nc.dram_tensor("wi", [128, KK * 2 * FF], F8, kind="ExternalInput")
    wf = nc.dram_tensor("wf", [JJ, 128, 2 * D], F8, kind="ExternalInput")
    cvec = nc.dram_tensor("cvec", [D], F32, kind="ExternalInput")
    g1 = nc.dram_tensor("g1", [D], F32, kind="ExternalInput")
    b1 = nc.dram_tensor("b1", [D], F32, kind="ExternalInput")
    g2 = nc.dram_tensor("g2", [D], F32, kind="ExternalInput")
    b2 = nc.dram_tensor("b2", [D], F32, kind="ExternalInput")
    bfv = nc.dram_tensor("bfv", [D], F32, kind="ExternalInput")
    bi = nc.dram_tensor("bi", [FF], F32, kind="ExternalInput")
    y = nc.dram_tensor("y", [TPC, D], F32, kind="ExternalOutput")

    with tile.TileContext(nc) as tc:
        with (
            tc.tile_pool(name="pw", bufs=1) as pw,
            tc.tile_pool(name="px", bufs=2 * NT) as px,
            tc.tile_pool(name="pbig", bufs=1) as pbig,
            tc.tile_pool(name="pabf", bufs=4) as pabf,
            tc.tile_pool(name="pstat", bufs=4) as pstat,
            tc.tile_pool(name="pout", bufs=3) as pout,
            tc.tile_pool(name="ps1", bufs=2, space="PSUM") as ps1,
            tc.tile_pool(name="psm", bufs=3, space="PSUM") as psm,
        ):
            # x tiles first: LN1(t0) heads the one-shot critical chain
            x_pre = []
            for t in range(NT):
                x_t = px.tile([128, D], F32, tag="xa")
                nc.sync.dma_start(out=x_t[:], in_=x[t * 128:(t + 1) * 128, :])
                x_pre.append(x_t)

            # broadcast constants go on the (parallel) SWDGE queue
            def bcast(handle, n, tag):
                t = pw.tile([128, n], F32, tag=tag)
                nc.gpsimd.dma_start(out=t[:], in_=_bcast_ap(handle, n))
                return t

            cvec_b = bcast(cvec, D, "cvec")
            g1_b = bcast(g1, D, "g1") if flags["g1b1"] else None
            b1_b = bcast(b1, D, "b1") if flags["g1b1"] else None
            g2_b = bcast(g2, D, "g2") if flags["g2b2"] else None
            b2_b = bcast(b2, D, "b2") if flags["g2b2"] else None
            bfv_b = bcast(bfv, D, "bfv") if flags["bfv"] else None
            bi_sb = None
            if flags["bi"]:
                # bi as [128, 24]: column m holds bi[m*128 : (m+1)*128]
                bi_sb = pw.tile([128, MF], F32, tag="bi")
                nc.gpsimd.dma_start(
                    out=bi_sb[:],
                    in_=bass.AP(tensor=bi, offset=0, ap=[[1, 128], [128, MF]]))
            scratch = pw.tile([128, 1], F32, tag="scratch")
            nc.vector.memset(scratch[:], 0.0)
            # dummy Gelu: hoists the one ACT function-table load into the
            # DMA prologue where it is fully hidden
            nc.scalar.activation(out=scratch[:], in_=scratch[:], func=AF.Gelu,
                                 scale=1.0)

            # weight DMAs go after the x tiles on the SP ring; the prep
            # transposes ride the ACT ring so they never wait behind these
            wi8_sb = pw.tile([128, KK * 2 * FF], F8, tag="wi8")
            nc.sync.dma_start(out=wi8_sb[:], in_=wi[:, :])
            wf8_tiles = []
            for jj in range(JJ):
                wt = pw.tile([128, 2 * D], F8, tag=f"wf{jj}")
                wf8_tiles.append(wt)
                nc.sync.dma_start(out=wt[:], in_=wf[jj])

            tensors = (x, y, wi8_sb, wf8_tiles, cvec_b, g1_b, b1_b,
                       g2_b, b2_b, bfv_b, bi_sb)
            pools = (pw, px, pbig, pabf, pstat, pout, ps1, psm)
            em = _Emitter(nc, pools, tensors, flags)

            genA, genB = _Gen("A"), _Gen("B")
            em.gen_alloc(genA)
            em.gen_alloc(genB)
            for t in range(NT):
                em.prep_tile(genA, t, x_t=x_pre[t])

            if isinstance(n_reps, tuple):  # ("loop", n) -> dynamic Tile loop
                assert n_reps[1] % 2 == 0
                with tc.For_i(0, n_reps[1] // 2, 1):
                    em.rep(genA, genB)
                    em.rep(genB, genA)
            else:
                gens = [genA, genB]
                for i in range(n_reps):
                    cur = gens[i % 2]
                    nxt = gens[(i + 1) % 2] if i + 1 < n_reps else None
                    em.rep(cur, nxt)

    nc.compile()
    _NC_CACHE[cache_key] = nc
    return nc


def _prep_inputs(hidden_states, Wo, bo, ln1_g, ln1_b, Wi, bi, Wf, bf,
                 ln2_g, ln2_b):
    x = np.ascontiguousarray(np.asarray(hidden_states, np.float32)
                             .reshape(TOK, D))
    Wo = np.asarray(Wo, np.float32)
    Wi = np.asarray(Wi, np.float32)
    Wf = np.asarray(Wf, np.float32)
    cvec = (0.01 * Wo.sum(axis=1) + np.asarray(bo, np.float32)).astype(np.float32)
    # wi layout [p, kk, i, f] = Wi.T[kk*256 + 2p + i, f] * S_W  (fp8)
    wi_s = (Wi.T * S_W).astype(np.float32)          # [D, FF]
    wi_prep = np.ascontiguousarray(
        wi_s.reshape(KK, 128, 2, FF).transpose(1, 0, 2, 3)
        .reshape(128, KK * 2 * FF).astype(F8NP))
    # wf layout [jj, p, i, d] = Wf.T[(2jj + i)*128 + p, d] * S_W  (fp8)
    wf_s = (Wf.T * S_W).astype(np.float32)          # [FF, D]
    wf_prep = np.ascontiguousarray(
        wf_s.reshape(JJ, 2, 128, D).transpose(0, 2, 1, 3)
        .reshape(JJ, 128, 2 * D).astype(F8NP))
    common = {
        "wi": wi_prep, "wf": wf_prep, "cvec": cvec,
        "g1": np.asarray(ln1_g, np.float32),
        "b1": np.asarray(ln1_b, np.float32) * np.float32(S_W),
        "g2": np.asarray(ln2_g, np.float32), "b2": np.asarray(ln2_b, np.float32),
        "bfv": np.asarray(bf, np.float32) * np.float32(S_W),
        "bi": np.asarray(bi, np.float32),
    }
    in_maps = [dict(common, x=x[c * TPC:(c + 1) * TPC]) for c in range(NCORE)]
    flag_key = (
        not (np.all(ln1_g == 1.0) and np.all(ln1_b == 0.0)),
        not (np.all(ln2_g == 1.0) and np.all(ln2_b == 0.0)),
        bool(np.any(np.asarray(bi) != 0.0)),
        bool(np.any(np.asarray(bf) != 0.0)),
    )
    return in_maps, flag_key


def kernel(hidden_states, Wq, bq, Wk, bk, Wv, bv, Wo, bo, ln1_g, ln1_b,
           Wi, bi, Wf, bf, ln2_g, ln2_b):
    global LAST_RESULTS
    B, S, _ = hidden_states.shape
    in_maps, flag_key = _prep_inputs(hidden_states, Wo, bo, ln1_g, ln1_b,
                                     Wi, bi, Wf, bf, ln2_g, ln2_b)
    nc = _build(RUN_KWARGS.get("n_reps", 1), flag_key)
    res = run_bass_kernel_spmd(nc, in_maps, list(range(NCORE)),
                               **{k: v for k, v in RUN_KWARGS.items()
                                  if k != "n_reps"})
    LAST_RESULTS = res
    out = np.concatenate([res.results[c]["y"] for c in range(NCORE)], axis=0)
    return np.ascontiguousarray(out.reshape(B, S, D).astype(np.float32))


# revision 9
# speedup vs baseline: 1.4274x; 1.4274x over previous
"""Trainium2 Bass kernel for nn_DebugBertLayer_87093346828840.

Key observation: the reference overwrites q/k/v with the constant 0.01, so
softmax(scores) is uniform and ctx == 0.01 everywhere.  Hence
    attn_out = LN1(hidden + cvec),   cvec = 0.01 * Wo.sum(axis=1) + bo
and the only real device work is the FFN:
    out = LN2( gelu(attn_out @ Wi.T + bi) @ Wf.T + bf + attn_out )

Sharding: pure data-parallel over the 8192 tokens -> 1024 tokens/core on 8
NeuronCores, no collectives.

Matmuls run in fp8e4 (e4m3) with perf_mode=DoubleRow: 2 fp8 weights/PE cell,
2 MACs/cycle, contraction 256/instruction -> ~1.8x the bf16 PE throughput.
Scale plumbing (all folded into existing ops, zero extra passes):
  - weights quantized at x256 host-side (keeps |w|~0.02 out of fp8 subnormals)
  - LN1's rsqrt is scaled by 256, so a_tiles hold 256*attn_out (fp32)
  - the fp8 cast of a_tiles uses scale 1/8 -> activations at x32
  - mm1 psum = 8192*h_pre; ACT Gelu drains it with scale=1/8192 -> h exact,
    written straight to fp8 (scale 1)
  - mm2 psum = 256*(h@Wf.T); the residual add uses the pre-scaled a_tiles and
    LayerNorm is scale-invariant, so LN2 absorbs the x256 for free.
DoubleRow pairing: contraction index (p, i) -> feature 2p+i (adjacent pairs),
so a [128,128]-uint16 DMA-xbar transpose of packed fp8 pairs produces exactly
the [p][i=2][tok] moving-operand layout mm1 needs.

Cross-rep software pipelining (the timing loop runs 2 reps per For_i
iteration, ping-ponging A/B buffer generations):
  body = mm1(A) ; mm2(A) with prep(B) interleaved tile-by-tile ;
         mm1(B) ; mm2(B) with prep(A') interleaved
prep(gen, t) = x DMA + cvec add + LN1 + fp8 cast (DVE) + 3 uint16 DMA-xbar
transposes (on the ACT HWDGE ring, so they never queue behind x/y traffic on
the SP ring).  The DVE ops of prep(next) are emitted interleaved with mm2's
drain+LN2 per token tile, so the Vector engine's FIFO alternates between
draining this rep and preparing the next one; the PE goes straight from
mm2(A) into mm1(B) with zero boundary bubble.
"""

import os
import sys

for _p in ("/opt/trn_rl_repo", "/root/.axon_site/_ro/trn_rl_repo"):
    if os.path.isdir(_p) and _p not in sys.path:
        sys.path.insert(0, _p)

import numpy as np
import ml_dtypes

import concourse.bass as bass
import concourse.bacc as bacc
import concourse.tile as tile
from concourse import mybir
from concourse.bass_utils import run_bass_kernel_spmd

F32 = mybir.dt.float32
BF16 = mybir.dt.bfloat16
F8 = mybir.dt.float8e4
U16 = mybir.dt.uint16
AF = mybir.ActivationFunctionType
ALU = mybir.AluOpType
DR = mybir.MatmulPerfMode.DoubleRow
F8NP = mybir.dt.np(mybir.dt.float8e4)   # ml_dtypes.float8_e4m3

D = 768           # d_model
FF = 3072         # d_ff
NCORE = 8
TOK = 8192        # total tokens (4 x 2048)
TPC = TOK // NCORE  # 1024 tokens per core
KK = D // 256     # 3 DoubleRow k-tiles over d_model
MF = FF // 128    # 24 tiles over d_ff
JJ = FF // 256    # 12 DoubleRow ff-tiles
NT = TPC // 128   # 8 token tiles per core
HALF = TPC // 2   # 512
LN_EPS = 1e-12

S_W = 256.0       # weight quant scale
S_A = 32.0        # activation quant scale (cast scale = S_A / S_W = 1/8)

_NC_CACHE = {}
LAST_RESULTS = None
RUN_KWARGS = {}


def _ln_tile(nc, pstat, s_t, gb, apply_gb, post_scale=1.0):
    """In-place LayerNorm over the free dim (768) of s_t [128, 768] f32.

    rstd is computed on the Vector engine (bit-trick seed + 2 Newton steps)
    instead of ScalarE Sqrt: the ACT engine then only ever runs Gelu, which
    avoids ~1.3-7.6us activation-table reloads on every Sqrt<->Gelu switch.
    post_scale is folded into rstd, so the output is post_scale*LN(x).
    var+eps == var in fp32 here (var ~1 >> 1e-12), so eps is skipped.
    """
    g_b, b_b = gb
    sr = s_t.rearrange("p (n s) -> p n s", s=384)
    stats = pstat.tile([128, 2, 6], F32, tag="stats")
    for i in range(2):
        nc.vector.bn_stats(out=stats[:, i, :], in_=sr[:, i, :])
    mv = pstat.tile([128, 2], F32, tag="mv")
    nc.vector.bn_aggr(out=mv[:], in_=stats[:])
    v = mv[:, 1:2]
    rst = pstat.tile([128, 1], F32, tag="rst")
    nrt = pstat.tile([128, 1], F32, tag="nrt")
    # y0 = bitcast(0x5f3759df - (bits(v) >> 1)): ~3.4% rsqrt seed
    nc.vector.tensor_scalar(out=rst.bitcast(mybir.dt.int32)[:],
                            in0=v.bitcast(mybir.dt.int32),
                            scalar1=1, scalar2=None,
                            op0=ALU.logical_shift_right)
    nc.vector.tensor_scalar(out=rst.bitcast(mybir.dt.int32)[:],
                            in0=rst.bitcast(mybir.dt.int32)[:],
                            scalar1=-1, scalar2=0x5F3759DF,
                            op0=ALU.mult, op1=ALU.add)
    for it in range(2):  # y <- y*(1.5 - 0.5*v*y^2); 2 steps -> ~1e-6 rel
        nc.vector.tensor_mul(out=nrt[:], in0=rst[:], in1=rst[:])
        nc.vector.tensor_mul(out=nrt[:], in0=nrt[:], in1=v)
        nc.vector.tensor_scalar(out=nrt[:], in0=nrt[:], scalar1=-0.5,
                                scalar2=1.5, op0=ALU.mult, op1=ALU.add)
        if it == 1 and post_scale != 1.0:
            # fold post_scale into the last Newton step's nrt factor
            nc.vector.tensor_scalar(out=nrt[:], in0=nrt[:],
                                    scalar1=post_scale, scalar2=None,
                                    op0=ALU.mult)
        nc.vector.tensor_mul(out=rst[:], in0=rst[:], in1=nrt[:])
    nc.vector.tensor_scalar(out=s_t[:], in0=s_t[:], scalar1=mv[:, 0:1],
                            scalar2=rst[:], op0=ALU.subtract, op1=ALU.mult)
    if apply_gb:
        nc.vector.tensor_mul(out=s_t[:], in0=s_t[:], in1=g_b[:])
        nc.vector.tensor_add(out=s_t[:], in0=s_t[:], in1=b_b[:])


class _Gen:
    """One generation of prepared activations (a_tiles + transposed fp8)."""

    def __init__(self, name):
        self.name = name
        self.a_tiles = [None] * NT
        self.aT8_u16 = None
        self.aT_r = None
        self.hT = None


class _Emitter:
    def __init__(self, nc, pools, tensors, flags):
        self.nc = nc
        (self.pw, self.px, self.pbig, self.pabf, self.pstat, self.pout,
         self.ps1, self.psm) = pools
        (self.x, self.y, self.wi8_sb, self.wf8_tiles, self.cvec_b,
         self.g1_b, self.b1_b, self.g2_b, self.b2_b, self.bfv_b,
         self.bi_sb) = tensors
        self.flags = flags
        self.wi8_r = self.wi8_sb.rearrange("p (c i f) -> p c i f", c=KK, i=2)
        self.wf_r = [w.rearrange("p (i d) -> p i d", i=2)
                     for w in self.wf8_tiles]

    def gen_alloc(self, gen):
        # fp8 [p][c][tok][i]: u16 view column c*TPC+t packs the
        # adjacent-feature pair (2p, 2p+1) of double-k-tile c at token t
        aT8 = self.pbig.tile([128, KK * TPC * 2], F8, tag=f"aT{gen.name}")
        gen.aT8_u16 = aT8.bitcast(U16)   # [128, KK*TPC]
        gen.aT_r = aT8.rearrange("p (c t i) -> p c i t", c=KK, i=2)
        gen.hT = self.pbig.tile([128, MF * TPC], F8, tag=f"hT{gen.name}")

    def prep_x(self, gen, t):
        x_t = self.px.tile([128, D], F32, tag="xa")
        self.nc.sync.dma_start(out=x_t[:],
                               in_=self.x[t * 128:(t + 1) * 128, :])
        gen.a_tiles[t] = x_t

    def prep_tile(self, gen, t, x_t=None):
        """cvec + LN1(*S_W) + fp8 cast (DVE) + 3 u16 DMA transposes."""
        nc = self.nc
        if x_t is None:
            x_t = gen.a_tiles[t]
        nc.vector.tensor_add(out=x_t[:], in0=x_t[:], in1=self.cvec_b[:])
        _ln_tile(nc, self.pstat, x_t, (self.g1_b, self.b1_b),
                 self.flags["g1b1"], post_scale=S_W)
        gen.a_tiles[t] = x_t
        a8 = self.pabf.tile([128, D], F8, tag="a8")
        nc.vector.tensor_scalar(out=a8[:], in0=x_t[:],
                                scalar1=S_A / S_W, scalar2=None,
                                op0=ALU.mult)
        a8u = a8.bitcast(U16)  # [128, 384] feature pairs
        for c in range(KK):
            dst = gen.aT8_u16[:, c * TPC + t * 128: c * TPC + (t + 1) * 128]
            # ACT's HWDGE ring: keeps transposes off the x/y SP ring
            nc.scalar.dma_start(out=dst, in_=a8u[:, c * 128:(c + 1) * 128],
                                transpose=True)

    def mm1(self, gen, hT, nxt=None):
        nc = self.nc
        if nxt is not None:
            # next generation's x loads: the SP ring is otherwise idle here
            for t in range(NT):
                self.prep_x(nxt, t)
        step = 0
        for ph in range(2):
            off = ph * HALF
            for m in range(MF):
                ps_a = self.ps1.tile([128, HALF], F32, tag="hps")
                for c in range(KK):
                    lhsT = self.wi8_r[:, c, :, m * 128:(m + 1) * 128]
                    nc.tensor.matmul(ps_a[:], lhsT,
                                     gen.aT_r[:, c, :, off:off + HALF],
                                     start=(c == 0), stop=(c == KK - 1),
                                     perf_mode=DR)
                dst = hT[:, m * TPC + off: m * TPC + off + HALF]
                if self.flags["bi"]:
                    nc.scalar.activation(out=dst, in_=ps_a[:], func=AF.Gelu,
                                         bias=self.bi_sb[:, m:m + 1],
                                         scale=1.0 / (S_A * S_W))
                else:
                    nc.scalar.activation(out=dst, in_=ps_a[:], func=AF.Gelu,
                                         scale=1.0 / (S_A * S_W))
                step += 1
                # one prep tile per 6 (ph, m)-steps: the prep DVE ladder runs
                # under mm1's matmuls (mm1 has no DVE work of its own)
                if nxt is not None and step % 6 == 0:
                    self.prep_tile(nxt, step // 6 - 1)

    def mm2(self, gen, hT, prep_cb=None):
        nc = self.nc
        # stationary hT slice [p][i=2][tok]: ff = (2*jj + i)*128 + p
        hT_r = hT.rearrange("p (j i t) -> p j i t", j=JJ, i=2)
        for t in range(NT):
            ps2 = self.psm.tile([128, D], F32, tag="psm")
            for jj in range(JJ):
                lhsT = hT_r[:, jj, :, t * 128:(t + 1) * 128]
                nc.tensor.matmul(ps2[:, 0:512], lhsT, self.wf_r[jj][:, :, 0:512],
                                 start=(jj == 0), stop=(jj == JJ - 1),
                                 perf_mode=DR)
                nc.tensor.matmul(ps2[:, 512:768], lhsT,
                                 self.wf_r[jj][:, :, 512:768],
                                 start=(jj == 0), stop=(jj == JJ - 1),
                                 perf_mode=DR)
            s_t = self.pout.tile([128, D], F32, tag="s")
            nc.vector.tensor_add(out=s_t[:], in0=ps2[:],
                                 in1=gen.a_tiles[t][:])
            if self.flags["bfv"]:
                nc.vector.tensor_add(out=s_t[:], in0=s_t[:], in1=self.bfv_b[:])
            _ln_tile(nc, self.pstat, s_t, (self.g2_b, self.b2_b),
                     self.flags["g2b2"])
            nc.sync.dma_start(out=self.y[t * 128:(t + 1) * 128, :],
                              in_=s_t[:])
            if prep_cb is not None:
                prep_cb(t)

    def rep(self, gen, nxt):
        """mm1+mm2 on `gen`; prep of `nxt` interleaved into mm1 (or None)."""
        self.mm1(gen, gen.hT, nxt=nxt)
        self.mm2(gen, gen.hT)


def _bcast_ap(handle, n):
    """AP that broadcasts a [n]-vector across 128 partitions for DMA."""
    return bass.AP(tensor=handle, offset=0, ap=[[0, 128], [1, n]])


def _build(n_reps=1, flag_key=(True, True, True, True)):
    cache_key = (n_reps, flag_key)
    if cache_key in _NC_CACHE:
        return _NC_CACHE[cache_key]
    flags = dict(zip(("g1b1", "g2b2", "bi", "bfv"), flag_key))
    nc = bacc.Bacc("TRN2", target_bir_lowering=False, debug=False,
                   num_devices=NCORE)
    x = nc.dram_tensor("x", [TPC, D], F32, kind="ExternalInput")
    wi = nc.dram_tensor("wi", [128, KK * 2 * FF], F8, kind="ExternalInput")
    wf = nc.dram_tensor("wf", [JJ, 128, 2 * D], F8, kind="ExternalInput")
    cvec = nc.dram_tensor("cvec", [D], F32, kind="ExternalInput")
    g1 = nc.dram_tensor("g1", [D], F32, kind="ExternalInput")
    b1 = nc.dram_tensor("b1", [D], F32, kind="ExternalInput")
    g2 = nc.dram_tensor("g2", [D], F32, kind="ExternalInput")
    b2 = nc.dram_tensor("b2", [D], F32, kind="ExternalInput")
    bfv = nc.dram_tensor("bfv", [D], F32, kind="ExternalInput")
    bi = nc.dram_tensor("bi", [FF], F32, kind="ExternalInput")
    y = nc.dram_tensor("y", [TPC, D], F32, kind="ExternalOutput")

    with tile.TileContext(nc) as tc:
        with (
            tc.tile_pool(name="pw", bufs=1) as pw,
            tc.tile_pool(name="px", bufs=2 * NT) as px,
            tc.tile_pool(name="pbig", bufs=1) as pbig,
            tc.tile_pool(name="pabf", bufs=4) as pabf,
            tc.tile_pool(name="pstat", bufs=4) as pstat,
            tc.tile_pool(name="pout", bufs=3) as pout,
            tc.tile_pool(name="ps1", bufs=2, space="PSUM") as ps1,
            tc.tile_pool(name="psm", bufs=3, space="PSUM") as psm,
        ):
            # x tiles first: LN1(t0) heads the one-shot critical chain
            x_pre = []
            for t in range(NT):
                x_t = px.tile([128, D], F32, tag="xa")
                nc.sync.dma_start(out=x_t[:], in_=x[t * 128:(t + 1) * 128, :])
                x_pre.append(x_t)

            # broadcast constants go on the (parallel) SWDGE queue
            def bcast(handle, n, tag):
                t = pw.tile([128, n], F32, tag=tag)
                nc.gpsimd.dma_start(out=t[:], in_=_bcast_ap(handle, n))
                return t

            cvec_b = bcast(cvec, D, "cvec")
            g1_b = bcast(g1, D, "g1") if flags["g1b1"] else None
            b1_b = bcast(b1, D, "b1") if flags["g1b1"] else None
            g2_b = bcast(g2, D, "g2") if flags["g2b2"] else None
            b2_b = bcast(b2, D, "b2") if flags["g2b2"] else None
            bfv_b = bcast(bfv, D, "bfv") if flags["bfv"] else None
            bi_sb = None
            if flags["bi"]:
                # bi as [128, 24]: column m holds bi[m*128 : (m+1)*128]
                bi_sb = pw.tile([128, MF], F32, tag="bi")
                nc.gpsimd.dma_start(
                    out=bi_sb[:],
                    in_=bass.AP(tensor=bi, offset=0, ap=[[1, 128], [128, MF]]))
            scratch = pw.tile([128, 1], F32, tag="scratch")
            nc.vector.memset(scratch[:], 0.0)
            # dummy Gelu: hoists the one ACT function-table load into the
            # DMA prologue where it is fully hidden
            nc.scalar.activation(out=scratch[:], in_=scratch[:], func=AF.Gelu,
                                 scale=1.0)

            # weight DMAs go after the x tiles on the SP ring; the prep
            # transposes ride the ACT ring so they never wait behind these
            wi8_sb = pw.tile([128, KK * 2 * FF], F8, tag="wi8")
            nc.sync.dma_start(out=wi8_sb[:], in_=wi[:, :])
            wf8_tiles = []
            for jj in range(JJ):
                wt = pw.tile([128, 2 * D], F8, tag=f"wf{jj}")
                wf8_tiles.append(wt)
                nc.sync.dma_start(out=wt[:], in_=wf[jj])

            tensors = (x, y, wi8_sb, wf8_tiles, cvec_b, g1_b, b1_b,
                       g2_b, b2_b, bfv_b, bi_sb)
            pools = (pw, px, pbig, pabf, pstat, pout, ps1, psm)
            em = _Emitter(nc, pools, tensors, flags)

            genA, genB = _Gen("A"), _Gen("B")
            em.gen_alloc(genA)
            em.gen_alloc(genB)
            for t in range(NT):
                em.prep_tile(genA, t, x_t=x_pre[t])

            if isinstance(n_reps, tuple):  # ("loop", n) -> dynamic Tile loop
                assert n_reps[1] % 2 == 0
                with tc.For_i(0, n_reps[1] // 2, 1):
                    em.rep(genA, genB)
                    em.rep(genB, genA)
            else:
                gens = [genA, genB]
                for i in range(n_reps):
                    cur = gens[i % 2]
                    nxt = gens[(i + 1) % 2] if i + 1 < n_reps else None
                    em.rep(cur, nxt)

    nc.compile()
    _NC_CACHE[cache_key] = nc
    return nc


def _prep_inputs(hidden_states, Wo, bo, ln1_g, ln1_b, Wi, bi, Wf, bf,
                 ln2_g, ln2_b):
    x = np.ascontiguousarray(np.asarray(hidden_states, np.float32)
                             .reshape(TOK, D))
    Wo = np.asarray(Wo, np.float32)
    Wi = np.asarray(Wi, np.float32)
    Wf = np.asarray(Wf, np.float32)
    cvec = (0.01 * Wo.sum(axis=1) + np.asarray(bo, np.float32)).astype(np.float32)
    # wi layout [p, kk, i, f] = Wi.T[kk*256 + 2p + i, f] * S_W  (fp8)
    wi_s = (Wi.T * S_W).astype(np.float32)          # [D, FF]
    wi_prep = np.ascontiguousarray(
        wi_s.reshape(KK, 128, 2, FF).transpose(1, 0, 2, 3)
        .reshape(128, KK * 2 * FF).astype(F8NP))
    # wf layout [jj, p, i, d] = Wf.T[(2jj + i)*128 + p, d] * S_W  (fp8)
    wf_s = (Wf.T * S_W).astype(np.float32)          # [FF, D]
    wf_prep = np.ascontiguousarray(
        wf_s.reshape(JJ, 2, 128, D).transpose(0, 2, 1, 3)
        .reshape(JJ, 128, 2 * D).astype(F8NP))
    common = {
        "wi": wi_prep, "wf": wf_prep, "cvec": cvec,
        "g1": np.asarray(ln1_g, np.float32),
        "b1": np.asarray(ln1_b, np.float32) * np.float32(S_W),
        "g2": np.asarray(ln2_g, np.float32), "b2": np.asarray(ln2_b, np.float32),
        "bfv": np.asarray(bf, np.float32) * np.float32(S_W),
        "bi": np.asarray(bi, np.float32),
    }
    in_maps = [dict(common, x=x[c * TPC:(c + 1) * TPC]) for c in range(NCORE)]
    flag_key = (
        not (np.all(ln1_g == 1.0) and np.all(ln1_b == 0.0)),
        not (np.all(ln2_g == 1.0) and np.all(ln2_b == 0.0)),
        bool(np.any(np.asarray(bi) != 0.0)),
        bool(np.any(np.asarray(bf) != 0.0)),
    )
    return in_maps, flag_key


def kernel(hidden_states, Wq, bq, Wk, bk, Wv, bv, Wo, bo, ln1_g, ln1_b,
           Wi, bi, Wf, bf, ln2_g, ln2_b):
    global LAST_RESULTS
    B, S, _ = hidden_states.shape
    in_maps, flag_key = _prep_inputs(hidden_states, Wo, bo, ln1_g, ln1_b,
                                     Wi, bi, Wf, bf, ln2_g, ln2_b)
    nc = _build(RUN_KWARGS.get("n_reps", 1), flag_key)
    res = run_bass_kernel_spmd(nc, in_maps, list(range(NCORE)),
                               **{k: v for k, v in RUN_KWARGS.items()
                                  if k != "n_reps"})
    LAST_RESULTS = res
    out = np.concatenate([res.results[c]["y"] for c in range(NCORE)], axis=0)
    return np.ascontiguousarray(out.reshape(B, S, D).astype(np.float32))
